# revision 29
# baseline (speedup 1.0000x reference)
"""ADM-Softmax (additive-margin softmax logits) distributed Bass kernel for
one TRN2 chip (8 NeuronCores).

Math (reference):
    kn   = weight / ||weight||_col            # [D, C], norm over D
    fn   = feats  / ||feats||_row             # [B, D], norm over D
    cos  = clip(fn @ kn, -1, 1)               # [B, C]  (clip inactive: |cos| < 0.3 for this regime)
    out  = (cos - margin[b] * onehot(labels[b]))[b, c] * 5.0
    margin[b] = 0.4 if labels[b] == 0 else 0.1

Sharding: columns (num_class C) split across 8 cores; feats/labels
replicated. C is zero-padded 100000 -> 102400 so each core owns 12800
columns. The SPMD graph is identical on all cores; everything
label-dependent is input data.

Per-core kernel:
  - weight shard arrives bf16, host-retiled to [10, 128, 4, 1280] so each
    column tile is one fully-contiguous 1.28 MB DMA (10 KB/partition)
  - feats arrive f32 [512, 512]; row norms on-device (ACT square+accum,
    sqrt, DVE reciprocal), normalized, transposed on the PE -> fnT bf16
  - per 128-column block: Gram matmul (w.T @ w, diagonal = column sumsq)
    on the PE; diag extracted with one DVE scalar_tensor_tensor against
    an identity mask; sqrt/reciprocal batched 10 blocks at a time and
    scheduled during the main matmul phase
  - main matmul out[c_blk, b] = w.T @ fnT accumulates 4 K-chunks in PSUM;
    the column scale 5/sqrt(sumsq+eps) is applied on the PSUM->SBUF copy
    (alternating ScalarE/VectorE), output bf16 [C_local, B], host
    transposes/concats/upcasts
  - margin: 4 indirect scatter-add DMAs add -5*margin[b] at the 512
    (c_local, b) positions (value 0.0 on non-owner cores -> no-op)
"""

import numpy as np
import ml_dtypes

from concourse import bacc, bass, mybir, tile
from concourse.bass import IndirectOffsetOnAxis
from concourse.bass_utils import run_bass_kernel_spmd

B = 512
D = 512
C = 100000
NCORES = 8
CPAD = 102400
CLOC = CPAD // NCORES          # 12800 columns per core
CTW = 1280                     # columns per DMA tile
NCT = CLOC // CTW              # 10 DMA tiles
NCS = CTW // 128               # 10 column blocks per DMA tile
P = 128
MARGIN_R = 0.4
MARGIN_F = 0.1
SCALE = 5.0
EPS = 1e-12

FP32 = mybir.dt.float32
BF16 = mybir.dt.bfloat16
I32 = mybir.dt.int32
AF = mybir.ActivationFunctionType
ALU = mybir.AluOpType

_CACHE = {}


def _build(margin_fix=True):
    nc = bacc.Bacc(
        "TRN2", target_bir_lowering=False, debug=False, num_devices=NCORES
    )
    w_ext = nc.dram_tensor("w", [NCT, P, 4, CTW], BF16, kind="ExternalInput")
    f_ext = nc.dram_tensor("feats", [B, D], FP32, kind="ExternalInput")
    id_ext = nc.dram_tensor("ident", [P, P], FP32, kind="ExternalInput")
    offs_ext = nc.dram_tensor("offs", [P, B // P], I32, kind="ExternalInput")
    madd_ext = nc.dram_tensor("madd", [P, B // P], BF16, kind="ExternalInput")
    out_ext = nc.dram_tensor("out", [CLOC * B, 1], BF16, kind="ExternalOutput")

    with tile.TileContext(nc) as tc:
        with (
            tc.tile_pool(name="constp", bufs=1) as constp,
            tc.tile_pool(name="fpool", bufs=1) as fpool,
            tc.tile_pool(name="wpool", bufs=4) as wpool,
            tc.tile_pool(name="opool", bufs=4) as opool,
            tc.tile_pool(name="spool", bufs=4) as spool,
            tc.tile_pool(name="psA", bufs=4, space="PSUM") as psA,
            tc.tile_pool(name="psB", bufs=4, space="PSUM") as psB,
        ):
            ident = constp.tile([P, P], FP32)
            nc.sync.dma_start(ident[:], id_ext[:])
            epsb = constp.tile([P, 1], FP32, tag="epsb")
            nc.gpsimd.memset(epsb[:], EPS)
            epsb2 = constp.tile([P, 1], FP32, tag="epsb2")
            nc.gpsimd.memset(epsb2[:], EPS / (SCALE * SCALE))
            offs = constp.tile([P, B // P], I32, tag="offs")
            nc.sync.dma_start(offs[:], offs_ext[:])
            madd = constp.tile([P, B // P], BF16, tag="madd")
            nc.sync.dma_start(madd[:], madd_ext[:])

            # ---- feats: row-normalize and transpose to fnT [d, b] bf16 ----
            fnT = constp.tile([P, 4, B], BF16, tag="fnT")
            for bt in range(4):
                f_t = fpool.tile([P, D], FP32, tag="f_t")
                nc.sync.dma_start(f_t[:], f_ext[bt * P:(bt + 1) * P, :])
                fsq = fpool.tile([P, D], FP32, tag="fsq")
                ssf = spool.tile([P, 1], FP32, tag="ssf")
                nc.scalar.activation(fsq[:], f_t[:], AF.Square, accum_out=ssf[:])
                tf = spool.tile([P, 1], FP32, tag="tf")
                nc.scalar.activation(tf[:], ssf[:], AF.Sqrt, bias=epsb[:])
                invf = spool.tile([P, 1], FP32, tag="invf")
                nc.vector.reciprocal(invf[:], tf[:])
                fn = fpool.tile([P, D], FP32, tag="fn")
                nc.scalar.activation(fn[:], f_t[:], AF.Copy, scale=invf[:])
                for dc in range(4):
                    pt = psB.tile([P, P], FP32, tag="pg")
                    nc.tensor.transpose(pt[:], fn[:, dc * P:(dc + 1) * P], ident[:])
                    nc.vector.tensor_copy(fnT[:, dc, bt * P:(bt + 1) * P], pt[:])

            # ---- main loop over column DMA tiles ----
            for ct in range(NCT):
                wt = wpool.tile([P, 4, CTW], BF16, tag="wt")
                # two half-tile DMAs: column blocks of the first half become
                # available while the second half is still in flight
                half = CTW // 2
                nc.sync.dma_start(wt[:, :, :half], w_ext[ct][:, :, :half])
                nc.sync.dma_start(wt[:, :, half:], w_ext[ct][:, :, half:])

                # gram phase first: the sqrt/reciprocal chain for this
                # tile's column scales overlaps the main matmul phase
                ssw = spool.tile([P, NCS], FP32, tag="ssw")
                for cs in range(NCS):
                    pg = psB.tile([P, P], FP32, tag="pg")
                    for dc in range(4):
                        lw = wt[:, dc, cs * P:(cs + 1) * P]
                        nc.tensor.matmul(
                            pg[:], lw, lw, start=(dc == 0), stop=(dc == 3)
                        )
                    scr = opool.tile([P, P], FP32, tag="scr")
                    nc.vector.scalar_tensor_tensor(
                        out=scr[:],
                        in0=pg[:],
                        scalar=1.0,
                        in1=ident[:],
                        op0=ALU.mult,
                        op1=ALU.mult,
                        accum_out=ssw[:, cs:cs + 1],
                    )
                tw = spool.tile([P, NCS], FP32, tag="tw")
                # tw = sqrt(ssw + EPS) / SCALE
                nc.scalar.activation(
                    tw[:],
                    ssw[:],
                    AF.Sqrt,
                    scale=1.0 / (SCALE * SCALE),
                    bias=epsb2[:],
                )
                sw = spool.tile([P, NCS], FP32, tag="sw")
                nc.vector.reciprocal(sw[:], tw[:])
                for cs in range(NCS):
                    po = psA.tile([P, B], FP32, tag="po")
                    for dc in range(4):
                        lw = wt[:, dc, cs * P:(cs + 1) * P]
                        nc.tensor.matmul(
                            po[:], lw, fnT[:, dc, :], start=(dc == 0), stop=(dc == 3)
                        )
                    ot = opool.tile([P, B], BF16, tag="ot")
                    if cs % 2 == 0:
                        nc.scalar.activation(
                            ot[:], po[:], AF.Copy, scale=sw[:, cs:cs + 1]
                        )
                    else:
                        nc.vector.tensor_scalar_mul(ot[:], po[:], sw[:, cs:cs + 1])
                    row0 = (ct * NCS + cs) * P
                    dst = out_ext[row0 * B:(row0 + P) * B, :].rearrange(
                        "(p b) one -> p (b one)", p=P
                    )
                    nc.sync.dma_start(dst, ot[:])

            # ---- margin: scatter-add -5*margin at the label positions ----
            if margin_fix:
                for j in range(B // P):
                    nc.gpsimd.indirect_dma_start(
                        out=out_ext[:],
                        out_offset=IndirectOffsetOnAxis(
                            ap=offs[:, j:j + 1], axis=0
                        ),
                        in_=madd[:, j:j + 1],
                        in_offset=None,
                        compute_op=ALU.add,
                    )
            else:
                dummy = constp.tile([P, B // P], BF16, tag="gat")
                nc.vector.tensor_copy(dummy[:], madd[:])
                idummy = constp.tile([P, B // P], I32, tag="fixed")
                nc.vector.tensor_copy(idummy[:], offs[:])

    nc.compile()
    return nc


def _get_nc():
    if "nc" not in _CACHE:
        _CACHE["nc"] = _build()
    return _CACHE["nc"]


def _prep_in_maps(feats, labels, weight):
    feats = np.ascontiguousarray(np.asarray(feats, dtype=np.float32))
    labels = np.asarray(labels).astype(np.int64)
    weight = np.asarray(weight, dtype=np.float32)

    wpad = np.zeros((D, CPAD), dtype=ml_dtypes.bfloat16)
    wpad[:, :C] = weight.astype(ml_dtypes.bfloat16)

    ident = np.eye(P, dtype=np.float32)

    c_local = (labels % CLOC).astype(np.int64)
    offs = (c_local * B + np.arange(B, dtype=np.int64)).astype(np.int32)
    offs = np.ascontiguousarray(offs.reshape(P, B // P))
    owner = (labels // CLOC).astype(np.int64)
    margin = np.where(labels == 0, MARGIN_R, MARGIN_F).astype(np.float32)

    in_maps = []
    for k in range(NCORES):
        wk = wpad[:, k * CLOC:(k + 1) * CLOC]
        # [D, CLOC] -> [NCT, P, 4, CTW]: w[dc*128+p, ct*CTW+cc]
        wk = np.ascontiguousarray(
            wk.reshape(4, P, NCT, CTW).transpose(2, 1, 0, 3)
        )
        madd = np.where(owner == k, -SCALE * margin, 0.0).astype(
            ml_dtypes.bfloat16
        )
        in_maps.append(
            {
                "w": wk,
                "feats": feats,
                "ident": ident,
                "offs": offs,
                "madd": np.ascontiguousarray(madd.reshape(P, B // P)),
            }
        )
    return in_maps


def _assemble(results):
    full = np.empty((B, CPAD), dtype=np.float32)
    for k in range(NCORES):
        out_k = results[k]["out"].reshape(CLOC, B).astype(np.float32)
        full[:, k * CLOC:(k + 1) * CLOC] = out_k.T
    return np.ascontiguousarray(full[:, :C])


def run(feats, labels, weight, trace=False, **spmd_kwargs):
    nc = _get_nc()
    in_maps = _prep_in_maps(feats, labels, weight)
    res = run_bass_kernel_spmd(
        nc, in_maps, core_ids=list(range(NCORES)), trace=trace, **spmd_kwargs
    )
    return _assemble(res.results), res


def kernel(feats, labels, weight):
    out, _ = run(feats, labels, weight)
    return out


# revision 30
# speedup vs baseline: 1.0011x; 1.0011x over previous
"""ADM-Softmax (additive-margin softmax logits) distributed Bass kernel for
one TRN2 chip (8 NeuronCores).

Math (reference):
    kn   = weight / ||weight||_col            # [D, C], norm over D
    fn   = feats  / ||feats||_row             # [B, D], norm over D
    cos  = clip(fn @ kn, -1, 1)               # [B, C]  (clip inactive: |cos| < 0.3 for this regime)
    out  = (cos - margin[b] * onehot(labels[b]))[b, c] * 5.0
    margin[b] = 0.4 if labels[b] == 0 else 0.1

Sharding: columns (num_class C) split across 8 cores; feats/labels
replicated. C is zero-padded 100000 -> 102400 so each core owns 12800
columns. The SPMD graph is identical on all cores; everything
label-dependent is input data.

Per-core kernel:
  - weight shard arrives bf16, host-retiled to [10, 128, 4, 1280] so each
    column tile is one fully-contiguous 1.28 MB DMA (10 KB/partition)
  - feats arrive f32 [512, 512]; row norms on-device (ACT square+accum,
    sqrt, DVE reciprocal), normalized, transposed on the PE -> fnT bf16
  - per 128-column block: Gram matmul (w.T @ w, diagonal = column sumsq)
    on the PE; diag extracted with one DVE scalar_tensor_tensor against
    an identity mask; sqrt/reciprocal batched 10 blocks at a time and
    scheduled during the main matmul phase
  - main matmul out[c_blk, b] = w.T @ fnT accumulates 4 K-chunks in PSUM;
    the column scale 5/sqrt(sumsq+eps) is applied on the PSUM->SBUF copy
    (alternating ScalarE/VectorE), output bf16 [C_local, B], host
    transposes/concats/upcasts
  - margin: 4 indirect scatter-add DMAs add -5*margin[b] at the 512
    (c_local, b) positions (value 0.0 on non-owner cores -> no-op)
"""

import numpy as np
import ml_dtypes

from concourse import bacc, bass, mybir, tile
from concourse.bass import IndirectOffsetOnAxis
from concourse.bass_utils import run_bass_kernel_spmd

B = 512
D = 512
C = 100000
NCORES = 8
CPAD = 102400
CLOC = CPAD // NCORES          # 12800 columns per core
CTW = 1280                     # columns per DMA tile
NCT = CLOC // CTW              # 10 DMA tiles
NCS = CTW // 128               # 10 column blocks per DMA tile
P = 128
MARGIN_R = 0.4
MARGIN_F = 0.1
SCALE = 5.0
EPS = 1e-12

FP32 = mybir.dt.float32
BF16 = mybir.dt.bfloat16
I32 = mybir.dt.int32
AF = mybir.ActivationFunctionType
ALU = mybir.AluOpType

_CACHE = {}


def _build(margin_fix=True):
    nc = bacc.Bacc(
        "TRN2", target_bir_lowering=False, debug=False, num_devices=NCORES
    )
    w_ext = nc.dram_tensor("w", [NCT, P, 4, CTW], BF16, kind="ExternalInput")
    f_ext = nc.dram_tensor("feats", [B, D], FP32, kind="ExternalInput")
    id_ext = nc.dram_tensor("ident", [P, P], FP32, kind="ExternalInput")
    offs_ext = nc.dram_tensor("offs", [P, B // P], I32, kind="ExternalInput")
    madd_ext = nc.dram_tensor("madd", [P, B // P], BF16, kind="ExternalInput")
    out_ext = nc.dram_tensor("out", [CLOC * B, 1], BF16, kind="ExternalOutput")

    with tile.TileContext(nc) as tc:
        with (
            tc.tile_pool(name="constp", bufs=1) as constp,
            tc.tile_pool(name="fpool", bufs=1) as fpool,
            tc.tile_pool(name="wpool", bufs=4) as wpool,
            tc.tile_pool(name="opool", bufs=4) as opool,
            tc.tile_pool(name="spool", bufs=4) as spool,
            tc.tile_pool(name="psA", bufs=4, space="PSUM") as psA,
            tc.tile_pool(name="psB", bufs=4, space="PSUM") as psB,
        ):
            ident = constp.tile([P, P], FP32)
            nc.sync.dma_start(ident[:], id_ext[:])
            epsb = constp.tile([P, 1], FP32, tag="epsb")
            nc.gpsimd.memset(epsb[:], EPS)
            epsb2 = constp.tile([P, 1], FP32, tag="epsb2")
            nc.gpsimd.memset(epsb2[:], EPS / (SCALE * SCALE))
            offs = constp.tile([P, B // P], I32, tag="offs")
            nc.sync.dma_start(offs[:], offs_ext[:])
            madd = constp.tile([P, B // P], BF16, tag="madd")
            nc.sync.dma_start(madd[:], madd_ext[:])

            # ---- feats: row-normalize and transpose to fnT [d, b] bf16 ----
            fnT = constp.tile([P, 4, B], BF16, tag="fnT")
            for bt in range(4):
                f_t = fpool.tile([P, D], FP32, tag="f_t")
                nc.sync.dma_start(f_t[:], f_ext[bt * P:(bt + 1) * P, :])
                fsq = fpool.tile([P, D], FP32, tag="fsq")
                ssf = spool.tile([P, 1], FP32, tag="ssf")
                nc.scalar.activation(fsq[:], f_t[:], AF.Square, accum_out=ssf[:])
                tf = spool.tile([P, 1], FP32, tag="tf")
                nc.scalar.activation(tf[:], ssf[:], AF.Sqrt, bias=epsb[:])
                invf = spool.tile([P, 1], FP32, tag="invf")
                nc.vector.reciprocal(invf[:], tf[:])
                fn = fpool.tile([P, D], FP32, tag="fn")
                nc.scalar.activation(fn[:], f_t[:], AF.Copy, scale=invf[:])
                for dc in range(4):
                    pt = psB.tile([P, P], FP32, tag="pg")
                    nc.tensor.transpose(pt[:], fn[:, dc * P:(dc + 1) * P], ident[:])
                    nc.vector.tensor_copy(fnT[:, dc, bt * P:(bt + 1) * P], pt[:])

            # ---- main loop over column DMA tiles ----
            for ct in range(NCT):
                wt = wpool.tile([P, 4, CTW], BF16, tag="wt")
                nc.sync.dma_start(wt[:], w_ext[ct])

                # gram phase first: the sqrt/reciprocal chain for this
                # tile's column scales overlaps the main matmul phase
                ssw = spool.tile([P, NCS], FP32, tag="ssw")
                for cs in range(NCS):
                    pg = psB.tile([P, P], FP32, tag="pg")
                    for dc in range(4):
                        lw = wt[:, dc, cs * P:(cs + 1) * P]
                        nc.tensor.matmul(
                            pg[:], lw, lw, start=(dc == 0), stop=(dc == 3)
                        )
                    scr = opool.tile([P, P], FP32, tag="scr")
                    nc.vector.scalar_tensor_tensor(
                        out=scr[:],
                        in0=pg[:],
                        scalar=1.0,
                        in1=ident[:],
                        op0=ALU.mult,
                        op1=ALU.mult,
                        accum_out=ssw[:, cs:cs + 1],
                    )
                tw = spool.tile([P, NCS], FP32, tag="tw")
                # tw = sqrt(ssw + EPS) / SCALE
                nc.scalar.activation(
                    tw[:],
                    ssw[:],
                    AF.Sqrt,
                    scale=1.0 / (SCALE * SCALE),
                    bias=epsb2[:],
                )
                sw = spool.tile([P, NCS], FP32, tag="sw")
                nc.vector.reciprocal(sw[:], tw[:])
                for cs in range(NCS):
                    po = psA.tile([P, B], FP32, tag="po")
                    for dc in range(4):
                        lw = wt[:, dc, cs * P:(cs + 1) * P]
                        nc.tensor.matmul(
                            po[:], lw, fnT[:, dc, :], start=(dc == 0), stop=(dc == 3)
                        )
                    ot = opool.tile([P, B], BF16, tag="ot")
                    if cs % 2 == 0:
                        nc.scalar.activation(
                            ot[:], po[:], AF.Copy, scale=sw[:, cs:cs + 1]
                        )
                    else:
                        nc.vector.tensor_scalar_mul(ot[:], po[:], sw[:, cs:cs + 1])
                    row0 = (ct * NCS + cs) * P
                    dst = out_ext[row0 * B:(row0 + P) * B, :].rearrange(
                        "(p b) one -> p (b one)", p=P
                    )
                    nc.sync.dma_start(dst, ot[:])

            # ---- margin: scatter-add -5*margin at the label positions ----
            if margin_fix:
                for j in range(B // P):
                    nc.gpsimd.indirect_dma_start(
                        out=out_ext[:],
                        out_offset=IndirectOffsetOnAxis(
                            ap=offs[:, j:j + 1], axis=0
                        ),
                        in_=madd[:, j:j + 1],
                        in_offset=None,
                        compute_op=ALU.add,
                    )
            else:
                dummy = constp.tile([P, B // P], BF16, tag="gat")
                nc.vector.tensor_copy(dummy[:], madd[:])
                idummy = constp.tile([P, B // P], I32, tag="fixed")
                nc.vector.tensor_copy(idummy[:], offs[:])

    nc.compile()
    return nc


def _get_nc():
    if "nc" not in _CACHE:
        _CACHE["nc"] = _build()
    return _CACHE["nc"]


def _prep_in_maps(feats, labels, weight):
    feats = np.ascontiguousarray(np.asarray(feats, dtype=np.float32))
    labels = np.asarray(labels).astype(np.int64)
    weight = np.asarray(weight, dtype=np.float32)

    wpad = np.zeros((D, CPAD), dtype=ml_dtypes.bfloat16)
    wpad[:, :C] = weight.astype(ml_dtypes.bfloat16)

    ident = np.eye(P, dtype=np.float32)

    c_local = (labels % CLOC).astype(np.int64)
    offs = (c_local * B + np.arange(B, dtype=np.int64)).astype(np.int32)
    offs = np.ascontiguousarray(offs.reshape(P, B // P))
    owner = (labels // CLOC).astype(np.int64)
    margin = np.where(labels == 0, MARGIN_R, MARGIN_F).astype(np.float32)

    in_maps = []
    for k in range(NCORES):
        wk = wpad[:, k * CLOC:(k + 1) * CLOC]
        # [D, CLOC] -> [NCT, P, 4, CTW]: w[dc*128+p, ct*CTW+cc]
        wk = np.ascontiguousarray(
            wk.reshape(4, P, NCT, CTW).transpose(2, 1, 0, 3)
        )
        madd = np.where(owner == k, -SCALE * margin, 0.0).astype(
            ml_dtypes.bfloat16
        )
        in_maps.append(
            {
                "w": wk,
                "feats": feats,
                "ident": ident,
                "offs": offs,
                "madd": np.ascontiguousarray(madd.reshape(P, B // P)),
            }
        )
    return in_maps


def _assemble(results):
    full = np.empty((B, CPAD), dtype=np.float32)
    for k in range(NCORES):
        out_k = results[k]["out"].reshape(CLOC, B).astype(np.float32)
        full[:, k * CLOC:(k + 1) * CLOC] = out_k.T
    return np.ascontiguousarray(full[:, :C])


def run(feats, labels, weight, trace=False, **spmd_kwargs):
    nc = _get_nc()
    in_maps = _prep_in_maps(feats, labels, weight)
    res = run_bass_kernel_spmd(
        nc, in_maps, core_ids=list(range(NCORES)), trace=trace, **spmd_kwargs
    )
    return _assemble(res.results), res


def kernel(feats, labels, weight):
    out, _ = run(feats, labels, weight)
    return out


# revision 31
# speedup vs baseline: 1.0917x; 1.0905x over previous
"""ADM-Softmax (additive-margin softmax logits) distributed Bass kernel for
one TRN2 chip (8 NeuronCores).

Math (reference):
    kn   = weight / ||weight||_col            # [D, C], norm over D
    fn   = feats  / ||feats||_row             # [B, D], norm over D
    cos  = clip(fn @ kn, -1, 1)               # [B, C]  (clip inactive: |cos| < 0.3 for this regime)
    out  = (cos - margin[b] * onehot(labels[b]))[b, c] * 5.0
    margin[b] = 0.4 if labels[b] == 0 else 0.1

Sharding: columns (num_class C) split across 8 cores; feats/labels
replicated. C is zero-padded 100000 -> 102400 so each core owns 12800
columns. The SPMD graph is identical on all cores; everything
label-dependent is input data.

Per-core kernel:
  - weight shard arrives bf16, host-retiled to [10, 128, 4, 1280] so each
    column tile is one fully-contiguous 1.28 MB DMA (10 KB/partition)
  - feats arrive f32 [512, 512]; row norms on-device (ACT square+accum,
    sqrt, DVE reciprocal), normalized, transposed on the PE -> fnT bf16
  - per 128-column block: Gram matmul (w.T @ w, diagonal = column sumsq)
    on the PE; diag extracted with one DVE scalar_tensor_tensor against
    an identity mask; sqrt/reciprocal batched 10 blocks at a time and
    scheduled during the main matmul phase
  - main matmul out[c_blk, b] = w.T @ fnT accumulates 4 K-chunks in PSUM;
    the column scale 5/sqrt(sumsq+eps) is applied on the PSUM->SBUF copy
    (alternating ScalarE/VectorE), output bf16 [C_local, B], host
    transposes/concats/upcasts
  - margin: 4 indirect scatter-add DMAs add -5*margin[b] at the 512
    (c_local, b) positions (value 0.0 on non-owner cores -> no-op)
"""

import numpy as np
import ml_dtypes

from concourse import bacc, bass, mybir, tile
from concourse.bass import IndirectOffsetOnAxis
from concourse.bass_utils import run_bass_kernel_spmd

B = 512
D = 512
C = 100000
NCORES = 8
CPAD = 102400
CLOC = CPAD // NCORES          # 12800 columns per core
CTW = 1280                     # columns per DMA tile
NCT = CLOC // CTW              # 10 DMA tiles
NCS = CTW // 128               # 10 column blocks per DMA tile
P = 128
MARGIN_R = 0.4
MARGIN_F = 0.1
SCALE = 5.0
EPS = 1e-12

FP32 = mybir.dt.float32
BF16 = mybir.dt.bfloat16
I32 = mybir.dt.int32
AF = mybir.ActivationFunctionType
ALU = mybir.AluOpType

_CACHE = {}


def _build(margin_fix=True):
    nc = bacc.Bacc(
        "TRN2", target_bir_lowering=False, debug=False, num_devices=NCORES
    )
    w_ext = nc.dram_tensor("w", [NCT, P, 4, CTW], BF16, kind="ExternalInput")
    f_ext = nc.dram_tensor("feats", [B, D], FP32, kind="ExternalInput")
    id_ext = nc.dram_tensor("ident", [P, P], FP32, kind="ExternalInput")
    offs_ext = nc.dram_tensor("offs", [P, B // P], I32, kind="ExternalInput")
    madd_ext = nc.dram_tensor("madd", [P, B // P], BF16, kind="ExternalInput")
    out_ext = nc.dram_tensor("out", [CLOC * B, 1], BF16, kind="ExternalOutput")

    with tile.TileContext(nc) as tc:
        with (
            tc.tile_pool(name="constp", bufs=1) as constp,
            tc.tile_pool(name="fpool", bufs=1) as fpool,
            tc.tile_pool(name="wpool", bufs=3) as wpool,
            tc.tile_pool(name="opool", bufs=4) as opool,
            tc.tile_pool(name="spool", bufs=4) as spool,
            tc.tile_pool(name="psA", bufs=4, space="PSUM") as psA,
            tc.tile_pool(name="psB", bufs=4, space="PSUM") as psB,
        ):
            ident = constp.tile([P, P], FP32)
            nc.sync.dma_start(ident[:], id_ext[:])
            epsb = constp.tile([P, 1], FP32, tag="epsb")
            nc.gpsimd.memset(epsb[:], EPS)
            epsb2 = constp.tile([P, 1], FP32, tag="epsb2")
            nc.gpsimd.memset(epsb2[:], EPS / (SCALE * SCALE))
            offs = constp.tile([P, B // P], I32, tag="offs")
            nc.sync.dma_start(offs[:], offs_ext[:])
            madd = constp.tile([P, B // P], BF16, tag="madd")
            nc.sync.dma_start(madd[:], madd_ext[:])

            # ---- feats: row-normalize and transpose to fnT [d, b] bf16 ----
            fnT = constp.tile([P, 4, B], BF16, tag="fnT")
            for bt in range(4):
                f_t = fpool.tile([P, D], FP32, tag="f_t")
                nc.sync.dma_start(f_t[:], f_ext[bt * P:(bt + 1) * P, :])
                fsq = fpool.tile([P, D], FP32, tag="fsq")
                ssf = spool.tile([P, 1], FP32, tag="ssf")
                nc.scalar.activation(fsq[:], f_t[:], AF.Square, accum_out=ssf[:])
                tf = spool.tile([P, 1], FP32, tag="tf")
                nc.scalar.activation(tf[:], ssf[:], AF.Sqrt, bias=epsb[:])
                invf = spool.tile([P, 1], FP32, tag="invf")
                nc.vector.reciprocal(invf[:], tf[:])
                fn = fpool.tile([P, D], FP32, tag="fn")
                nc.scalar.activation(fn[:], f_t[:], AF.Copy, scale=invf[:])
                for dc in range(4):
                    pt = psB.tile([P, P], FP32, tag="pg")
                    nc.tensor.transpose(pt[:], fn[:, dc * P:(dc + 1) * P], ident[:])
                    nc.vector.tensor_copy(fnT[:, dc, bt * P:(bt + 1) * P], pt[:])

            # ---- main loop over column DMA tiles ----
            for ct in range(NCT):
                wt = wpool.tile([P, 4, CTW], BF16, tag="wt")
                nc.sync.dma_start(wt[:], w_ext[ct])

                # gram phase first: the sqrt/reciprocal chain for this
                # tile's column scales overlaps the main matmul phase
                ssw = spool.tile([P, NCS], FP32, tag="ssw")
                for cs in range(NCS):
                    pg = psB.tile([P, P], FP32, tag="pg")
                    for dc in range(4):
                        lw = wt[:, dc, cs * P:(cs + 1) * P]
                        nc.tensor.matmul(
                            pg[:], lw, lw, start=(dc == 0), stop=(dc == 3)
                        )
                    scr = opool.tile([P, P], FP32, tag="scr")
                    nc.vector.scalar_tensor_tensor(
                        out=scr[:],
                        in0=pg[:],
                        scalar=1.0,
                        in1=ident[:],
                        op0=ALU.mult,
                        op1=ALU.mult,
                        accum_out=ssw[:, cs:cs + 1],
                    )
                tw = spool.tile([P, NCS], FP32, tag="tw")
                # tw = sqrt(ssw + EPS) / SCALE
                nc.scalar.activation(
                    tw[:],
                    ssw[:],
                    AF.Sqrt,
                    scale=1.0 / (SCALE * SCALE),
                    bias=epsb2[:],
                )
                sw = spool.tile([P, NCS], FP32, tag="sw")
                nc.vector.reciprocal(sw[:], tw[:])
                for cs in range(NCS):
                    po = psA.tile([P, B], FP32, tag="po")
                    for dc in range(4):
                        lw = wt[:, dc, cs * P:(cs + 1) * P]
                        nc.tensor.matmul(
                            po[:], lw, fnT[:, dc, :], start=(dc == 0), stop=(dc == 3)
                        )
                    ot = opool.tile([P, B], BF16, tag="ot")
                    if cs % 2 == 0:
                        nc.scalar.activation(
                            ot[:], po[:], AF.Copy, scale=sw[:, cs:cs + 1]
                        )
                    else:
                        nc.vector.tensor_scalar_mul(ot[:], po[:], sw[:, cs:cs + 1])
                    row0 = (ct * NCS + cs) * P
                    dst = out_ext[row0 * B:(row0 + P) * B, :].rearrange(
                        "(p b) one -> p (b one)", p=P
                    )
                    nc.sync.dma_start(dst, ot[:])

            # ---- margin: scatter-add -5*margin at the label positions ----
            if margin_fix:
                for j in range(B // P):
                    nc.gpsimd.indirect_dma_start(
                        out=out_ext[:],
                        out_offset=IndirectOffsetOnAxis(
                            ap=offs[:, j:j + 1], axis=0
                        ),
                        in_=madd[:, j:j + 1],
                        in_offset=None,
                        compute_op=ALU.add,
                    )
            else:
                dummy = constp.tile([P, B // P], BF16, tag="gat")
                nc.vector.tensor_copy(dummy[:], madd[:])
                idummy = constp.tile([P, B // P], I32, tag="fixed")
                nc.vector.tensor_copy(idummy[:], offs[:])

    nc.compile()
    return nc


def _get_nc():
    if "nc" not in _CACHE:
        _CACHE["nc"] = _build()
    return _CACHE["nc"]


def _prep_in_maps(feats, labels, weight):
    feats = np.ascontiguousarray(np.asarray(feats, dtype=np.float32))
    labels = np.asarray(labels).astype(np.int64)
    weight = np.asarray(weight, dtype=np.float32)

    wpad = np.zeros((D, CPAD), dtype=ml_dtypes.bfloat16)
    wpad[:, :C] = weight.astype(ml_dtypes.bfloat16)

    ident = np.eye(P, dtype=np.float32)

    c_local = (labels % CLOC).astype(np.int64)
    offs = (c_local * B + np.arange(B, dtype=np.int64)).astype(np.int32)
    offs = np.ascontiguousarray(offs.reshape(P, B // P))
    owner = (labels // CLOC).astype(np.int64)
    margin = np.where(labels == 0, MARGIN_R, MARGIN_F).astype(np.float32)

    in_maps = []
    for k in range(NCORES):
        wk = wpad[:, k * CLOC:(k + 1) * CLOC]
        # [D, CLOC] -> [NCT, P, 4, CTW]: w[dc*128+p, ct*CTW+cc]
        wk = np.ascontiguousarray(
            wk.reshape(4, P, NCT, CTW).transpose(2, 1, 0, 3)
        )
        madd = np.where(owner == k, -SCALE * margin, 0.0).astype(
            ml_dtypes.bfloat16
        )
        in_maps.append(
            {
                "w": wk,
                "feats": feats,
                "ident": ident,
                "offs": offs,
                "madd": np.ascontiguousarray(madd.reshape(P, B // P)),
            }
        )
    return in_maps


def _assemble(results):
    full = np.empty((B, CPAD), dtype=np.float32)
    for k in range(NCORES):
        out_k = results[k]["out"].reshape(CLOC, B).astype(np.float32)
        full[:, k * CLOC:(k + 1) * CLOC] = out_k.T
    return np.ascontiguousarray(full[:, :C])


def run(feats, labels, weight, trace=False, **spmd_kwargs):
    nc = _get_nc()
    in_maps = _prep_in_maps(feats, labels, weight)
    res = run_bass_kernel_spmd(
        nc, in_maps, core_ids=list(range(NCORES)), trace=trace, **spmd_kwargs
    )
    return _assemble(res.results), res


def kernel(feats, labels, weight):
    out, _ = run(feats, labels, weight)
    return out
